# revision 4
# baseline (speedup 1.0000x reference)
"""MoE top-2 SwiGLU kernel for TRN2, expert-parallel across 8 NeuronCores.

Strategy:
  - Host: fp32 gating (softmax + top-2, exact replication of the reference),
    dispatch = gather each expert's tokens into a padded [d, C] activation
    block (expert parallelism: core e holds expert e's weights only).
  - Device (per core): bf16 SwiGLU MLP over that expert's tokens:
        h = silu(W1 @ x) * (W3 @ x);  out = W2 @ h
    computed entirely transposed ([feature, token] layout) so both matmul
    stages contract on the partition dim with zero on-device transposes.
  - Host: combine = scatter-add weighted expert outputs (fp32).
"""

import numpy as np
import ml_dtypes

import concourse.bass as bass
import concourse.bacc as bacc
import concourse.mybir as mybir
import concourse.tile as tile
from concourse.bass_utils import run_bass_kernel_spmd

BF16 = mybir.dt.bfloat16
F32 = mybir.dt.float32

NUM_EXPERTS = 8
TOP_K = 2
D_MODEL = 1024
D_MLP = 3584
KD = D_MODEL // 128  # 8 contraction chunks over d_model
FC = D_MLP // 128    # 28 chunks over d_mlp

# Populated after each kernel() call so test.py can report device timing.
LAST_RUN = {}

# Overridable for CoreSim checks (Silu not implemented in the interpreter).
ACT_FN = mybir.ActivationFunctionType.Silu


def _t_tiles(C):
    tiles = []
    t0 = 0
    while t0 < C:
        tn = min(512, C - t0)
        tiles.append((t0, tn))
        t0 += tn
    return tiles


def _build_bass(C):
    t_tiles = _t_tiles(C)
    nc = bacc.Bacc("TRN2", target_bir_lowering=False, debug=False,
                   num_devices=NUM_EXPERTS)

    xt_d = nc.dram_tensor("xt", [KD, 128, C], BF16, kind="ExternalInput")
    w1_d = nc.dram_tensor("w1t", [FC, 128, D_MODEL], BF16, kind="ExternalInput")
    w3_d = nc.dram_tensor("w3t", [FC, 128, D_MODEL], BF16, kind="ExternalInput")
    w2_d = nc.dram_tensor("w2t", [KD, 128, D_MLP], BF16, kind="ExternalInput")
    out_d = nc.dram_tensor("out", [KD, 128, C], F32, kind="ExternalOutput")

    with tile.TileContext(nc) as tc:
        with (
            tc.tile_pool(name="xpool", bufs=1) as xpool,
            tc.tile_pool(name="wpool", bufs=4) as wpool,
            tc.tile_pool(name="w2pool", bufs=2) as w2pool,
            tc.tile_pool(name="hpool", bufs=1) as hpool,
            tc.tile_pool(name="spool", bufs=4) as spool,
            tc.tile_pool(name="opool", bufs=4) as opool,
            tc.tile_pool(name="ps1", bufs=2, space="PSUM") as ps1,
            tc.tile_pool(name="ps2", bufs=2, space="PSUM") as ps2,
        ):
            # Resident activations: X^T as 8 chunks of [128 (d), C (tokens)].
            xts = []
            for kd in range(KD):
                t = xpool.tile([128, C], BF16, tag=f"xt{kd}")
                nc.sync.dma_start(t[:], xt_d[kd])
                xts.append(t)

            # Stage 1: h^T[fc] = silu(W1 x)^T * (W3 x)^T, per 128-row f chunk.
            hts = []
            for fc in range(FC):
                w1 = wpool.tile([128, D_MODEL], BF16, tag="w1")
                nc.sync.dma_start(w1[:], w1_d[fc])
                w3 = wpool.tile([128, D_MODEL], BF16, tag="w3")
                nc.sync.dma_start(w3[:], w3_d[fc])
                ht = hpool.tile([128, C], BF16, tag=f"h{fc}")
                for (t0, tn) in t_tiles:
                    p1 = ps1.tile([128, tn], F32, tag="p1")
                    p3 = ps1.tile([128, tn], F32, tag="p3")
                    for kd in range(KD):
                        nc.tensor.matmul(
                            p1[:], w1[:, kd * 128:(kd + 1) * 128],
                            xts[kd][:, t0:t0 + tn],
                            start=(kd == 0), stop=(kd == KD - 1))
                    for kd in range(KD):
                        nc.tensor.matmul(
                            p3[:], w3[:, kd * 128:(kd + 1) * 128],
                            xts[kd][:, t0:t0 + tn],
                            start=(kd == 0), stop=(kd == KD - 1))
                    s1 = spool.tile([128, tn], F32, tag="s")
                    nc.scalar.activation(s1[:], p1[:], ACT_FN)
                    nc.vector.tensor_mul(ht[:, t0:t0 + tn], s1[:], p3[:])
                hts.append(ht)

            # Stage 2: out^T[dc] = sum_fc W2T[fc,dc]^T @ h^T[fc]
            for dc in range(KD):
                w2 = w2pool.tile([128, D_MLP], BF16, tag="w2")
                nc.sync.dma_start(w2[:], w2_d[dc])
                for (t0, tn) in t_tiles:
                    po = ps2.tile([128, tn], F32, tag="po")
                    for fc in range(FC):
                        nc.tensor.matmul(
                            po[:], w2[:, fc * 128:(fc + 1) * 128],
                            hts[fc][:, t0:t0 + tn],
                            start=(fc == 0), stop=(fc == FC - 1))
                    ot = opool.tile([128, tn], F32, tag="o")
                    nc.vector.tensor_copy(ot[:], po[:])
                    nc.sync.dma_start(out_d[dc][:, t0:t0 + tn], ot[:])

    nc.compile()
    return nc


def _gate(xt, W_gate):
    """fp32 softmax top-2 gating, matching jax.lax.top_k tie-breaking."""
    logits = xt @ W_gate.T
    m = logits.max(-1, keepdims=True)
    ex = np.exp(logits - m)
    w = ex / ex.sum(-1, keepdims=True)
    top_i = np.argsort(-w, axis=-1, kind="stable")[:, :TOP_K]
    top_w = np.take_along_axis(w, top_i, -1)
    top_w = top_w / top_w.sum(-1, keepdims=True)
    return top_i, top_w.astype(np.float32)


def kernel(x, W_gate, W1, W3, W2):
    x = np.asarray(x, dtype=np.float32)
    W_gate = np.asarray(W_gate, dtype=np.float32)
    W1 = np.asarray(W1, dtype=np.float32)
    W3 = np.asarray(W3, dtype=np.float32)
    W2 = np.asarray(W2, dtype=np.float32)

    B, P, D = x.shape
    T = B * P
    xt = x.reshape(T, D)

    top_i, top_w = _gate(xt, W_gate)

    idxs, wts = [], []
    for e in range(NUM_EXPERTS):
        rows, slots = np.nonzero(top_i == e)
        idxs.append(rows)
        wts.append(top_w[rows, slots])

    C = max(512, -(-max(len(i) for i in idxs) // 128) * 128)

    bf = ml_dtypes.bfloat16
    in_maps = []
    for e in range(NUM_EXPERTS):
        XT = np.zeros((D, C), dtype=bf)
        n = len(idxs[e])
        XT[:, :n] = xt[idxs[e]].T.astype(bf)
        # lhsT tile layouts, pre-tiled on host so device DMAs are contiguous:
        # w1t[fc, dp, kd*128+fp] = W1[e][fc*128+fp, kd*128+dp]
        w1t = np.ascontiguousarray(
            W1[e].T.reshape(KD, 128, FC, 128).transpose(2, 1, 0, 3)
            .reshape(FC, 128, D_MODEL).astype(bf))
        w3t = np.ascontiguousarray(
            W3[e].T.reshape(KD, 128, FC, 128).transpose(2, 1, 0, 3)
            .reshape(FC, 128, D_MODEL).astype(bf))
        # w2t[dc, fp, fc*128+dp] = W2[e][dc*128+dp, fc*128+fp]
        w2t = np.ascontiguousarray(
            W2[e].T.reshape(FC, 128, KD, 128).transpose(2, 1, 0, 3)
            .reshape(KD, 128, D_MLP).astype(bf))
        in_maps.append({
            "xt": np.ascontiguousarray(XT.reshape(KD, 128, C)),
            "w1t": w1t, "w3t": w3t, "w2t": w2t,
        })

    nc = _build_bass(C)
    res = run_bass_kernel_spmd(nc, in_maps, list(range(NUM_EXPERTS)))
    LAST_RUN["results"] = res
    LAST_RUN["C"] = C
    LAST_RUN["nc"] = nc
    LAST_RUN["in_maps"] = in_maps

    out = np.zeros((T, D), dtype=np.float32)
    for e in range(NUM_EXPERTS):
        O = np.asarray(res.results[e]["out"]).reshape(D, C)
        n = len(idxs[e])
        if n:
            out[idxs[e]] += wts[e][:, None] * O[:, :n].T
    return out.reshape(B, P, D)


# revision 13
# speedup vs baseline: 4.3915x; 4.3915x over previous
"""MoE top-2 SwiGLU kernel for TRN2, expert-parallel across 8 NeuronCores.

Strategy:
  - Host: fp32 gating (softmax + top-2, exact replication of the reference),
    dispatch = gather each expert's tokens into a padded [d, C] activation
    block (expert parallelism: core e holds expert e's weights only).
  - Device (per core): bf16 SwiGLU MLP over that expert's tokens:
        h = silu(W1 @ x) * (W3 @ x);  out = W2 @ h
    computed entirely transposed ([feature, token] layout) so both matmul
    stages contract on the partition dim with zero on-device transposes.
  - Host: combine = scatter-add weighted expert outputs (fp32).
"""

import numpy as np
import ml_dtypes

import concourse.bass as bass
import concourse.bacc as bacc
import concourse.mybir as mybir
import concourse.tile as tile
from concourse.bass_utils import run_bass_kernel_spmd

BF16 = mybir.dt.bfloat16
F32 = mybir.dt.float32

NUM_EXPERTS = 8
TOP_K = 2
D_MODEL = 1024
D_MLP = 3584
KD = D_MODEL // 128  # 8 contraction chunks over d_model
FC = D_MLP // 128    # 28 chunks over d_mlp

# Populated after each kernel() call so test.py can report device timing.
LAST_RUN = {}

# Overridable for CoreSim checks (Silu not implemented in the interpreter).
ACT_FN = mybir.ActivationFunctionType.Silu

# Tunables (model-swept via TimelineSim; best: X_FIRST + PS2_BUFS=3).
PS1_BUFS = 2
PS2_BUFS = 3
W_BUFS = 4
W2_BUFS = 2
X_FIRST = True  # emit w1/w3 fc=0 DMAs before the xt loads
FC0_KD_OUTER = False  # first f-chunk: kd-outer MM order to overlap xt DMA


def _t_tiles(C):
    tiles = []
    t0 = 0
    while t0 < C:
        tn = min(512, C - t0)
        tiles.append((t0, tn))
        t0 += tn
    return tiles


def _build_bass(C):
    t_tiles = _t_tiles(C)
    nc = bacc.Bacc("TRN2", target_bir_lowering=False, debug=False,
                   num_devices=NUM_EXPERTS)

    xt_d = nc.dram_tensor("xt", [KD, 128, C], BF16, kind="ExternalInput")
    w1_d = nc.dram_tensor("w1t", [FC, 128, D_MODEL], BF16, kind="ExternalInput")
    w3_d = nc.dram_tensor("w3t", [FC, 128, D_MODEL], BF16, kind="ExternalInput")
    w2_d = nc.dram_tensor("w2t", [KD, 128, D_MLP], BF16, kind="ExternalInput")
    out_d = nc.dram_tensor("out", [KD, 128, C], F32, kind="ExternalOutput")

    with tile.TileContext(nc) as tc:
        with (
            tc.tile_pool(name="xpool", bufs=1) as xpool,
            tc.tile_pool(name="wpool", bufs=W_BUFS) as wpool,
            tc.tile_pool(name="w2pool", bufs=W2_BUFS) as w2pool,
            tc.tile_pool(name="hpool", bufs=1) as hpool,
            tc.tile_pool(name="spool", bufs=4) as spool,
            tc.tile_pool(name="opool", bufs=4) as opool,
            tc.tile_pool(name="ps1", bufs=PS1_BUFS, space="PSUM") as ps1,
            tc.tile_pool(name="ps2", bufs=PS2_BUFS, space="PSUM") as ps2,
        ):
            w1_first = w3_first = None
            if X_FIRST:
                w1_first = wpool.tile([128, D_MODEL], BF16, tag="w1")
                nc.sync.dma_start(w1_first[:], w1_d[0])
                w3_first = wpool.tile([128, D_MODEL], BF16, tag="w3")
                nc.sync.dma_start(w3_first[:], w3_d[0])

            # Resident activations: X^T as 8 chunks of [128 (d), C (tokens)].
            xts = []
            for kd in range(KD):
                t = xpool.tile([128, C], BF16, tag=f"xt{kd}")
                nc.sync.dma_start(t[:], xt_d[kd])
                xts.append(t)

            # Stage 1: h^T[fc] = silu(W1 x)^T * (W3 x)^T, per 128-row f chunk.
            hts = []
            for fc in range(FC):
                if fc == 0 and X_FIRST:
                    w1, w3 = w1_first, w3_first
                else:
                    w1 = wpool.tile([128, D_MODEL], BF16, tag="w1")
                    nc.sync.dma_start(w1[:], w1_d[fc])
                    w3 = wpool.tile([128, D_MODEL], BF16, tag="w3")
                    nc.sync.dma_start(w3[:], w3_d[fc])
                ht = hpool.tile([128, C], BF16, tag=f"h{fc}")
                head = []
                if fc == 0 and FC0_KD_OUTER:
                    # kd-outer interleave over the first two token tiles: each
                    # xt chunk is consumed right as its DMA lands instead of
                    # stalling the first psum group on all 8 chunks. Two live
                    # groups per tag fit PS1_BUFS=2.
                    head = t_tiles[:2]
                    ps = [(ps1.tile([128, tn], F32, tag="p1", name=f"p1k{t0}"),
                           ps1.tile([128, tn], F32, tag="p3", name=f"p3k{t0}"))
                          for (t0, tn) in head]
                    for kd in range(KD):
                        for (p1, p3), (t0, tn) in zip(ps, head):
                            nc.tensor.matmul(
                                p1[:], w1[:, kd * 128:(kd + 1) * 128],
                                xts[kd][:, t0:t0 + tn],
                                start=(kd == 0), stop=(kd == KD - 1))
                            nc.tensor.matmul(
                                p3[:], w3[:, kd * 128:(kd + 1) * 128],
                                xts[kd][:, t0:t0 + tn],
                                start=(kd == 0), stop=(kd == KD - 1))
                    for (p1, p3), (t0, tn) in zip(ps, head):
                        s1 = spool.tile([128, tn], F32, tag="s")
                        nc.scalar.activation(s1[:], p1[:], ACT_FN)
                        nc.vector.tensor_mul(ht[:, t0:t0 + tn], s1[:], p3[:])
                for (t0, tn) in t_tiles[len(head):]:
                    p1 = ps1.tile([128, tn], F32, tag="p1")
                    p3 = ps1.tile([128, tn], F32, tag="p3")
                    for kd in range(KD):
                        nc.tensor.matmul(
                            p1[:], w1[:, kd * 128:(kd + 1) * 128],
                            xts[kd][:, t0:t0 + tn],
                            start=(kd == 0), stop=(kd == KD - 1))
                    for kd in range(KD):
                        nc.tensor.matmul(
                            p3[:], w3[:, kd * 128:(kd + 1) * 128],
                            xts[kd][:, t0:t0 + tn],
                            start=(kd == 0), stop=(kd == KD - 1))
                    s1 = spool.tile([128, tn], F32, tag="s")
                    nc.scalar.activation(s1[:], p1[:], ACT_FN)
                    nc.vector.tensor_mul(ht[:, t0:t0 + tn], s1[:], p3[:])
                hts.append(ht)

            # Stage 2: out^T[dc] = sum_fc W2T[fc,dc]^T @ h^T[fc]
            for dc in range(KD):
                w2 = w2pool.tile([128, D_MLP], BF16, tag="w2")
                nc.sync.dma_start(w2[:], w2_d[dc])
                for (t0, tn) in t_tiles:
                    po = ps2.tile([128, tn], F32, tag="po")
                    for fc in range(FC):
                        nc.tensor.matmul(
                            po[:], w2[:, fc * 128:(fc + 1) * 128],
                            hts[fc][:, t0:t0 + tn],
                            start=(fc == 0), stop=(fc == FC - 1))
                    ot = opool.tile([128, tn], F32, tag="o")
                    nc.vector.tensor_copy(ot[:], po[:])
                    nc.sync.dma_start(out_d[dc][:, t0:t0 + tn], ot[:])

    nc.compile()
    return nc


def _gate(xt, W_gate):
    """fp32 softmax top-2 gating, matching jax.lax.top_k tie-breaking."""
    logits = xt @ W_gate.T
    m = logits.max(-1, keepdims=True)
    ex = np.exp(logits - m)
    w = ex / ex.sum(-1, keepdims=True)
    top_i = np.argsort(-w, axis=-1, kind="stable")[:, :TOP_K]
    top_w = np.take_along_axis(w, top_i, -1)
    top_w = top_w / top_w.sum(-1, keepdims=True)
    return top_i, top_w.astype(np.float32)


def kernel(x, W_gate, W1, W3, W2):
    x = np.asarray(x, dtype=np.float32)
    W_gate = np.asarray(W_gate, dtype=np.float32)
    W1 = np.asarray(W1, dtype=np.float32)
    W3 = np.asarray(W3, dtype=np.float32)
    W2 = np.asarray(W2, dtype=np.float32)

    B, P, D = x.shape
    T = B * P
    xt = x.reshape(T, D)

    top_i, top_w = _gate(xt, W_gate)

    idxs, wts = [], []
    for e in range(NUM_EXPERTS):
        rows, slots = np.nonzero(top_i == e)
        idxs.append(rows)
        wts.append(top_w[rows, slots])

    max_count = max(len(i) for i in idxs)
    # SBUF fits C up to ~2000 (h residency dominates); split into passes if a
    # pathological routing concentrates tokens on few experts.
    n_pass = max(1, -(-max_count // 1536))
    cap = -(-max_count // n_pass)
    C = max(512, -(-cap // 16) * 16)

    bf = ml_dtypes.bfloat16
    wt_maps = []
    for e in range(NUM_EXPERTS):
        # lhsT tile layouts, pre-tiled on host so device DMAs are contiguous:
        # w1t[fc, dp, kd*128+fp] = W1[e][fc*128+fp, kd*128+dp]
        w1t = np.ascontiguousarray(
            W1[e].T.reshape(KD, 128, FC, 128).transpose(2, 1, 0, 3)
            .reshape(FC, 128, D_MODEL).astype(bf))
        w3t = np.ascontiguousarray(
            W3[e].T.reshape(KD, 128, FC, 128).transpose(2, 1, 0, 3)
            .reshape(FC, 128, D_MODEL).astype(bf))
        # w2t[dc, fp, fc*128+dp] = W2[e][dc*128+dp, fc*128+fp]
        w2t = np.ascontiguousarray(
            W2[e].T.reshape(FC, 128, KD, 128).transpose(2, 1, 0, 3)
            .reshape(KD, 128, D_MLP).astype(bf))
        wt_maps.append({"w1t": w1t, "w3t": w3t, "w2t": w2t})

    nc = _build_bass(C)
    out = np.zeros((T, D), dtype=np.float32)
    for p in range(n_pass):
        in_maps = []
        for e in range(NUM_EXPERTS):
            sel = idxs[e][p * C:(p + 1) * C]
            XT = np.zeros((D, C), dtype=bf)
            XT[:, :len(sel)] = xt[sel].T.astype(bf)
            in_maps.append({
                "xt": np.ascontiguousarray(XT.reshape(KD, 128, C)),
                **wt_maps[e],
            })
        res = run_bass_kernel_spmd(nc, in_maps, list(range(NUM_EXPERTS)))
        LAST_RUN["results"] = res
        LAST_RUN["C"] = C
        LAST_RUN["nc"] = nc
        LAST_RUN["in_maps"] = in_maps
        for e in range(NUM_EXPERTS):
            sel = idxs[e][p * C:(p + 1) * C]
            if len(sel):
                O = np.asarray(res.results[e]["out"]).reshape(D, C)
                w_sel = wts[e][p * C:(p + 1) * C]
                out[sel] += w_sel[:, None] * O[:, :len(sel)].T
    return out.reshape(B, P, D)
